# revision 41
# baseline (speedup 1.0000x reference)
"""DN4 episodic kNN scoring kernel for Trainium2 (Bass/Tile).

Per episode t (one NeuronCore each):
  q:(75,640,100) s:(25,640,100) fp32
  qn = q / ||q||_hw (per (wq,c));  sn = s / ||s||_c (per support position)
  rel[wq,way] = qn[wq]^T @ sn[way]  (100x500)
  score[wq,way] = sum over 100 rows of (sum of top-3 of each row's 500)
Output per core: (375,) fp32 = scores in (wq, way) order.

Design (v13, ~301us HW vs 324us baseline; DVE max8 is the pacing engine):
  - fp8e4 DoubleRow matmuls (K=256/instr, contraction padded 640->768);
    per chunk the weight (query) loop is outer so 5 ways share ldweights
  - DVE runs the irreducible top-8 scans (295 x 500-elem max8, straight
    from PSUM) nearly exclusively; the per-chunk regroup matmul consumes
    max8's top-3 slots directly (15 moving cols into one accumulating
    PSUM tile; single final [75,5,3]->[75,5] reduce)
  - q/s are uploaded as f16 (halves DMA; they get squared into f16 and
    quantized to fp8 on device anyway)
  - qprep msq: scalar Square + DVE reduce, except kc 1,3 of the in-loop
    groups which use scalar Square+accum_out per wq to unload the DVE
  - support norm: ss reduced AND row-broadcast by one ones[128,128]
    stationary matmul per way; Ln/Exp on scalar (DVE reciprocal is
    iterative ~6.4 cyc/elem); SN8 scale muls on DVE (kc4 on gpsimd)
  - prologue: s loads via gpsimd software-DGE (generation-only queue
    cost) + sync, then mini-q (queries 0-4) so chunks 0-2 start as soon
    as SN8 lands; gpsimd ISA library preloaded with a dummy op; chunks
    0-2 emitted before group-1's finish so first max8s aren't queued
    behind its DVE reduces
"""

import math

import numpy as np

import concourse.bass as bass
import concourse.mybir as mybir
from concourse import bacc
from concourse.tile import TileContext
from concourse.bass_utils import run_bass_kernel_spmd

T, WQ, C, HW = 8, 75, 640, 100
WAY, SHOT, NK = 5, 5, 3
SP = SHOT * HW          # 500 support positions per way
NSP = WAY * SP          # 2500 total support positions
KC = C // 128           # 5 contraction chunks of 128
NP = 3                  # kc pairs for DoubleRow (kc 5 is a zero pad)
NPAIR = WQ * WAY        # 375 output scores per episode
WQ_GRP = 25             # query-prep group (divides 75)
NROW = WQ * HW          # 7500 packed query rows
NCH = (NROW + 127) // 128   # 59 row chunks
NROWP = NCH * 128       # 7552: row stride, 16B-aligned for dual-fp8 ldweights
SPP = 512               # per-way support stride (16B-aligned)

QSCALE = 16.0           # fp8 scale for normalized q
SSCALE = 32.0           # fp8 scale for normalized s
OSCALE = 1.0 / (QSCALE * SSCALE)   # 1/512, exact in fp16

f32 = mybir.dt.float32
f16 = mybir.dt.float16
f8 = mybir.dt.float8e4
u32 = mybir.dt.uint32
AF = mybir.ActivationFunctionType
AX = mybir.AxisListType
OP = mybir.AluOpType
DR = mybir.MatmulPerfMode.DoubleRow


def build_kernel(nc, tc, q_dram, s_dram, ind_dram, out_dram):
    from contextlib import ExitStack

    ctx = ExitStack()
    with ctx:
        const = ctx.enter_context(tc.tile_pool(name="const", bufs=1))
        sn_pool = ctx.enter_context(tc.tile_pool(name="sn", bufs=1))
        q8_pool = ctx.enter_context(tc.tile_pool(name="q8", bufs=1))
        misc = ctx.enter_context(tc.tile_pool(name="misc", bufs=1))
        qmini = ctx.enter_context(tc.tile_pool(name="qmini", bufs=5))
        qld = ctx.enter_context(tc.tile_pool(name="qld", bufs=5))
        qst = ctx.enter_context(tc.tile_pool(name="qst", bufs=4))
        sqd = ctx.enter_context(tc.tile_pool(name="sqd", bufs=3))

        # ---- gpsimd ISA library preload: a dummy op so the ~11us Q7
        # library load overlaps the DMA phase instead of the first scale
        scratch = const.tile([128, 8], f16, tag="scratch")
        nc.gpsimd.memset(scratch[:], 0.0)
        nc.gpsimd.tensor_add(scratch[:, 0:4], scratch[:, 0:4], scratch[:, 4:8])

        MINI_N = 5

        # ---- constants ----
        # ones [128,128] stationary: the ss matmul then reduces over c AND
        # broadcasts the result to all 128 partitions in one shot (the old
        # separate broadcast matmul + PSUM copies disappear)
        ones_kk = const.tile([128, 128], f16, tag="ones_kk")
        nc.vector.memset(ones_kk[:], 1.0)

        # persistent fp8 operands: [128, 2, free] per kc-pair (DoubleRow
        # layout). One tile per (pair, way): the dual-fp8 moving operand
        # only works at AP offset 0.
        SN8 = [[sn_pool.tile([128, 2, SPP], f8, tag=f"sn{j}_{w}",
                             name=f"sn{j}_{w}") for w in range(WAY)]
               for j in range(NP)]
        Q8 = [q8_pool.tile([128, 2, NROWP], f8, tag=f"q8_{j}", name=f"q8_{j}")
              for j in range(NP)]
        # zero the pad slice (kc=5), per-way pad columns, and the row tail
        # [NROW, NROWP). gpsimd memsets: DVE time is precious, gpsimd idles.
        nc.gpsimd.memset(Q8[NP - 1][:, 1, :].bitcast(u32), 0)
        for j in range(NP):
            for w in range(WAY):
                eng = nc.vector if (j * WAY + w) % 2 == 0 else nc.gpsimd
                eng.memset(SN8[j][w][:, :, :].bitcast(u32), 0)
        for j in range(NP):
            nc.gpsimd.memset(Q8[j][:, :, NROW:NROWP].bitcast(u32), 0)

        ind_sb = misc.tile([128, NCH * WQ], f16, tag="ind_sb")

        # ---------- query prep pieces ----------
        # msq[c, w] = sum_h qg[c, w, h]^2 via scalar Square + accum_out:
        # one [128, 100] activation per (kc, wq). The elementwise result
        # lands in a rotating dummy tile; only the accumulator is consumed.
        def qprep_load(wq0, nw, pool, tag):
            tiles = []
            for kc in range(KC):
                qg = pool.tile([128, nw * HW], f16, tag=tag, name=tag)
                nc.sync.dma_start(
                    qg[:].rearrange("c (w h) -> c w h", w=nw),
                    q_dram[wq0:wq0 + nw, kc * 128:(kc + 1) * 128, :]
                        .rearrange("w c h -> c w h"),
                )
                tiles.append(qg)
            return tiles

        def qprep_finish(wq0, nw, tiles, accum_kcs=(1, 3)):
            for kc in range(KC):
                qg = tiles[kc]
                msqf = qst.tile([128, WQ_GRP], f32, tag="msqf", name="msqf")
                if kc in accum_kcs:
                    # DVE is the loop bottleneck: offload this kc's sum of
                    # squares to scalar Square+accum_out (per-wq [128,100])
                    dummy = sqd.tile([128, HW], f16, tag="sqdump",
                                     name="sqdump")
                    for w in range(nw):
                        nc.scalar.activation(
                            dummy[:], qg[:, w * HW:(w + 1) * HW], AF.Square,
                            accum_out=msqf[:, w:w + 1],
                        )
                else:
                    sq = sqd.tile([128, WQ_GRP * HW], f16, tag="qsq",
                                  name="qsq")
                    nc.scalar.activation(sq[:, 0:nw * HW], qg[:, 0:nw * HW],
                                         AF.Square)
                    sq3 = sq[:, 0:nw * HW].rearrange("c (w h) -> c w h", w=nw)
                    with nc.allow_low_precision("f16 sum of 100 squares"):
                        nc.vector.tensor_reduce(
                            msqf[:, 0:nw], sq3, axis=AX.X, op=OP.add,
                        )
                # rq = QSCALE / sqrt(msq) = sqrt(QSCALE^2 * (1/msq))
                rcp = qst.tile([128, WQ_GRP], f32, tag="rcp", name="rcp")
                nc.vector.reciprocal(rcp[:, 0:nw], msqf[:, 0:nw])
                rq = qst.tile([128, WQ_GRP], f32, tag="rq", name="rq")
                nc.scalar.activation(
                    rq[:, 0:nw], rcp[:, 0:nw], AF.Sqrt, scale=QSCALE * QSCALE)
                nc.gpsimd.tensor_mul(
                    Q8[kc // 2][:, kc % 2, wq0 * HW:(wq0 + nw) * HW]
                        .rearrange("c (w h) -> c w h", w=nw),
                    qg[:, 0:nw * HW].rearrange("c (w h) -> c w h", w=nw),
                    rq[:, 0:nw].to_broadcast([128, nw, HW]),
                )

        def qprep(wq0, nw):
            qprep_finish(wq0, nw, qprep_load(wq0, nw, qld, "qg32"))

        # ================= support preparation =================
        # stage all 5 k-slices of s in SBUF (50KB/partition), compute
        # per-position column norms, then scale+convert into SN8.
        sctx = ExitStack()
        sprep = sctx.enter_context(tc.tile_pool(name="sprep", bufs=1))
        sqp = sctx.enter_context(tc.tile_pool(name="sqp", bufs=2))

        # spread the 5 big loads: gpsimd software-DGE only pays descriptor
        # generation on its queue (transfers run async on the DMA rings),
        # so it takes 4; the sync queue stays free for mini/ind/group loads.
        # s data gates the longest chain (sq->ss->norm->SN8) so it goes
        # before the mini query loads.
        s32s = []
        dma_engs = [nc.gpsimd, nc.gpsimd, nc.sync, nc.gpsimd, nc.gpsimd]
        for kc in range(KC):
            s32 = sprep.tile([128, NSP], f16, tag=f"s32_{kc}", name=f"s32_{kc}")
            dma_engs[kc].dma_start(
                s32[:].rearrange("c (w h) -> c w h", w=WAY * SHOT),
                s_dram[:, kc * 128:(kc + 1) * 128, :].rearrange("w c h -> c w h"),
            )
            s32s.append(s32)

        # mini query group (queries 0-4): needed for chunk 0, after s
        mini_qg = []
        for kc in range(KC):
            qg = qmini.tile([128, MINI_N * HW], f16, tag="qmg", name="qmg")
            nc.sync.dma_start(
                qg[:].rearrange("c (w h) -> c w h", w=MINI_N),
                q_dram[0:MINI_N, kc * 128:(kc + 1) * 128, :]
                    .rearrange("w c h -> c w h"),
            )
            mini_qg.append(qg)

        # ind needed by chunk 0's regroup matmul; queue it early
        nc.sync.dma_start(
            ind_sb[:].rearrange("p (n w) -> p n w", n=NCH),
            ind_dram.rearrange("n p w -> p n w"),
        )
        # group-1 query loads: sync queue, behind mini + s32 kc2
        g1_tiles = qprep_load(5, 20, qld, "qg32")

        # mini norm stats first: small scalar squares + DVE reduces land
        # before the big support squares on each engine's FIFO
        mini_rq = []
        for kc in range(KC):
            qg = mini_qg[kc]
            sq = sqd.tile([128, WQ_GRP * HW], f16, tag="qsq", name="qsq")
            nc.scalar.activation(sq[:, 0:MINI_N * HW], qg[:, 0:MINI_N * HW],
                                 AF.Square)
            msq = qst.tile([128, WQ_GRP], f16, tag="msq", name="msq")
            with nc.allow_low_precision("f16 sum of 100 squares"):
                nc.vector.tensor_reduce(
                    msq[:, 0:MINI_N],
                    sq[:, 0:MINI_N * HW].rearrange("c (w h) -> c w h", w=MINI_N),
                    axis=AX.X, op=OP.add,
                )
            rcp = qst.tile([128, WQ_GRP], f32, tag="rcp", name="rcp")
            nc.vector.reciprocal(rcp[:, 0:MINI_N], msq[:, 0:MINI_N])
            mini_rq.append(rcp)

        with tc.tile_pool(name="ss_psum", bufs=1, space="PSUM") as spsum:
            ss_ps = [spsum.tile([128, SP], f32, tag=f"ss{j}", name=f"ss{j}")
                     for j in range(WAY)]
            for kc in range(KC):
                sq = sqp.tile([128, NSP], f16, tag="sq", name=f"sq_{kc}")
                nc.scalar.activation(sq[:], s32s[kc][:], AF.Square)
                for j in range(WAY):
                    nc.tensor.matmul(
                        ss_ps[j][:],
                        ones_kk[:],
                        sq[:, j * SP:(j + 1) * SP],
                        start=(kc == 0), stop=(kc == KC - 1),
                    )
            # mini scale+convert: sqrt on scalar, muls on gpsimd — both
            # queue behind the support squares but before Ln/Exp
            for kc in range(KC):
                rq = qst.tile([128, WQ_GRP], f32, tag="rq", name="rq")
                nc.scalar.activation(
                    rq[:, 0:MINI_N], mini_rq[kc][:, 0:MINI_N],
                    AF.Sqrt, scale=QSCALE * QSCALE)
                nc.gpsimd.tensor_mul(
                    Q8[kc // 2][:, kc % 2, 0:MINI_N * HW]
                        .rearrange("c (w h) -> c w h", w=MINI_N),
                    mini_qg[kc][:, 0:MINI_N * HW]
                        .rearrange("c (w h) -> c w h", w=MINI_N),
                    rq[:, 0:MINI_N].to_broadcast([128, MINI_N, HW]),
                )
            # bc_sb = SSCALE / sqrt(ss) = exp(-0.5*ln(ss/SSCALE^2)), already
            # row-broadcast by the ones[128,128] stationary; Ln/Exp on the
            # scalar engine (DVE reciprocal is iterative, ~6.4 cyc/elem)
            lnv = misc.tile([128, NSP], f16, tag="lnv")
            bc_sb = misc.tile([128, NSP], f32, tag="bc_sb")
            for j in range(WAY):
                nc.scalar.activation(
                    lnv[:, j * SP:(j + 1) * SP], ss_ps[j][:], AF.Ln,
                    scale=1.0 / (SSCALE * SSCALE))
                nc.scalar.activation(
                    bc_sb[:, j * SP:(j + 1) * SP],
                    lnv[:, j * SP:(j + 1) * SP],
                    AF.Exp, scale=-0.5,
                )
            # SN8 scale+convert: DVE streams these at ~670ns; SN8[2] (kc4
            # only, tile-disjoint from DVE's) goes to gpsimd in parallel
            for kc in range(KC):
                for w in range(WAY):
                    eng = nc.gpsimd if kc == 4 else nc.vector
                    eng.tensor_mul(
                        SN8[kc // 2][w][:, kc % 2, 0:SP],
                        s32s[kc][:, w * SP:(w + 1) * SP],
                        bc_sb[:, w * SP:(w + 1) * SP],
                    )
        sctx.close()  # free staged support SBUF

        # ================= main loop: one 128-row chunk =================
        relp = ctx.enter_context(tc.tile_pool(name="rel_psum", bufs=7, space="PSUM"))
        finp = ctx.enter_context(tc.tile_pool(name="fin_psum", bufs=1, space="PSUM"))
        m8buf = ctx.enter_context(tc.tile_pool(name="m8buf", bufs=4))

        # fin2 accumulates [wq, (way, e<3)] over all chunks; one final reduce
        fin2 = finp.tile([WQ, WAY * NK], f32, tag="fin2")

        def main_chunk(c):
            c0 = c * 128
            mc = min(128, NROW - c0)
            rels = [relp.tile([128, SPP], f32, tag="rel", name=f"rel{w}")
                    for w in range(WAY)]
            # j-outer: one weight set per (chunk, pair) feeds all 5 ways
            for j in range(NP):
                for w in range(WAY):
                    nc.tensor.matmul(
                        rels[w][:, :],
                        Q8[j][:, :, c0:c0 + 128],
                        SN8[j][w][:, :, :],
                        start=(j == 0), stop=(j == NP - 1),
                        perf_mode=DR,
                    )
            m8 = m8buf.tile([128, WAY * 8], f16, tag="m8", name="m8")
            for w in range(WAY):
                nc.vector.max(out=m8[:mc, w * 8:(w + 1) * 8],
                              in_=rels[w][:mc, 0:SP])
            # regroup matmul reads the top-3 slots of m8 directly:
            # fin2[wq, w*3+e] += sum_rows ind[row, wq] * m8[row, w*8+e]
            m8v = m8[:mc, :].rearrange("p (w e) -> p w e", w=WAY)[:, :, 0:NK]
            nc.tensor.matmul(
                fin2[:],
                ind_sb[:mc, c * WQ:(c + 1) * WQ],
                m8v,
                start=(c == 0), stop=(c == NCH - 1),
            )

        # mini group covers chunks [0,3); chunks 0-2 are emitted BEFORE
        # group-1's normalization so the first max8s aren't queued behind
        # its 2.2us DVE reduces
        for c in range(0, 3):
            main_chunk(c)
        # group 1 has a short window (chunks 3-18): all-DVE stats; the
        # scalar accum path would gate its Q8 by ~30us
        qprep_finish(5, 20, g1_tiles, accum_kcs=())   # -> chunks [0,19)
        qprep(25, 25)      # -> chunks [0,39)
        for c in range(3, 19):
            main_chunk(c)
        qprep(50, 25)      # -> all chunks
        for c in range(19, 39):
            main_chunk(c)
        for c in range(39, NCH):
            main_chunk(c)

        out_sb = misc.tile([WQ, WAY], f32, tag="out_sb")
        nc.vector.tensor_reduce(
            out_sb[:],
            fin2[:].rearrange("p (w e) -> p w e", w=WAY),
            axis=AX.X, op=OP.add,
        )
        nc.sync.dma_start(out_dram.rearrange("(a b) -> a b", a=WQ), out_sb[:])


_CACHED = {}


def _make_ind():
    # indicator with the fp8 output scale folded in (1/512, exact in fp16)
    ind = np.zeros((NCH, 128, WQ), dtype=np.float16)
    rows = np.arange(NROW)
    for c in range(NCH):
        sel = rows[(rows >= c * 128) & (rows < (c + 1) * 128)]
        ind[c, sel - c * 128, sel // HW] = OSCALE
    return ind


def _get_compiled():
    if "nc" in _CACHED:
        return _CACHED["nc"]
    nc = bacc.Bacc(
        "TRN2", target_bir_lowering=False, debug=False,
        enable_asserts=False, num_devices=T,
    )
    q_dram = nc.dram_tensor("q", [WQ, C, HW], f16, kind="ExternalInput").ap()
    s_dram = nc.dram_tensor("s", [WAY * SHOT, C, HW], f16, kind="ExternalInput").ap()
    ind_dram = nc.dram_tensor("ind", [NCH, 128, WQ], f16, kind="ExternalInput").ap()
    out_dram = nc.dram_tensor("out", [NPAIR], f32, kind="ExternalOutput").ap()
    with TileContext(nc) as tc:
        build_kernel(nc, tc, q_dram, s_dram, ind_dram, out_dram)
    nc.compile()
    _CACHED["nc"] = nc
    return nc


def _make_in_maps(query_feat, support_feat):
    q = np.ascontiguousarray(
        np.asarray(query_feat, dtype=np.float32).reshape(T, WQ, C, HW)
    ).astype(np.float16)
    s = np.ascontiguousarray(
        np.asarray(support_feat, dtype=np.float32).reshape(T, WAY * SHOT, C, HW)
    ).astype(np.float16)
    ind = _make_ind()
    return [{"q": q[i], "s": s[i], "ind": ind} for i in range(T)]


def run(query_feat, support_feat):
    nc = _get_compiled()
    in_maps = _make_in_maps(query_feat, support_feat)
    res = run_bass_kernel_spmd(nc, in_maps, core_ids=list(range(T)))
    out = np.stack(
        [res.results[i]["out"].reshape(WQ, WAY) for i in range(T)], axis=0
    ).astype(np.float32)
    return out, res


def kernel(**inputs):
    out, _ = run(inputs["query_feat"], inputs["support_feat"])
    return out
